# revision 9
# baseline (speedup 1.0000x reference)
"""Trainium2 Bass kernel for nn_Decoder (2-layer LSTM autoregressive decoder).

Model (see reference): B=256 batch, T=512 steps, H=256 hidden, 2 LSTM layers,
scalar (F=1) autoregressive feedback through an output projection, final
output = scalar MSE loss vs `sequence`.

Strategy (v2)
-------------
- Data-parallel over batch: 8 NeuronCores x 32 batches each. Weights
  replicated. Zero cross-core communication; loss assembled on host.
- Per core, per step, gates are computed as [128, 256] PSUM tiles with
  partition = 32*gate + batch (gate order i,f,o,g) via 4-way column-tiled
  matmuls (M=32 stationary h, weights moving, all bf16).
- The scalar pred feedback is folded into a rank-1 matrix
  Am = 0.5*outer(W_out, W_ih_l0), so gates_l0(t) = H0(t-1)@A0 + H1(t-1)@Am
  + bias (a K=1 ones-row matmul).
- Wave order per step keeps only the truly h1(t-1)/h0(t)-dependent waves
  (Am, A1h, A1i) near the recurrence critical path; bias+A0 issue first and
  fill the PE under the previous step's cell-1 chain. (fp8 DoubleRow was
  evaluated to halve these waves but is invalid ISA for dst partitions
  32/96 - s3d3_mm_valid_dst_partition - so the waves stay bf16.)
- Cell chain per layer: one tanh act over all four gates
  (sigmoid(x) = (tanh(x/2)+1)/2, g-gate pre-activations doubled host-side,
  states kept doubled H=2h / S=2c), 2 PE transposes, then THREE DVE ops:
      copy  yg -> U[slot0]             (U[slot1] holds S persistently)
      PQ    = (Y[i,f] + 1) * U         (p = 2*sig(i)*g~ and q = 4*sig(f)*c
                                        fused into ONE scalar_tensor_tensor)
      S'    = 0.5*PQ[q] + PQ[p]        (written back into U[slot1])
  then tc = tanh(0.5*S') on the Act engine and H' = (yo+1)*tc.
- h1 history is written straight into a [128, T*64] bf16 SBUF buffer by the
  H' update op and DMA'd to HBM in chunks; preds + loss computed on host.
"""

import sys

import numpy as np

if "/opt/trn_rl_repo" not in sys.path:
    sys.path.insert(0, "/opt/trn_rl_repo")

import ml_dtypes

B, T, H = 256, 512, 256
NCORES = 8
BSH = B // NCORES  # 32 batches per core
NG = 4  # gates
GW = H  # gate width in j-dim (256)

BF16 = ml_dtypes.bfloat16

# device gate order: i, f, o, g (so sigmoid gates are partitions 0..95)
# pytorch row order in the 4H dim: i, f, g, o
_PERM = np.concatenate([
    np.arange(0, 256),       # i
    np.arange(256, 512),     # f
    np.arange(768, 1024),    # o
    np.arange(512, 768),     # g
])

# column offsets inside the big bf16 const block [128, CB_COLS]
_OFF_A0 = 0
_OFF_AM = 2048
_OFF_A1I = 4096
_OFF_A1H = 6144
_OFF_IDENT = 8192
_OFF_H0I = 8320
_OFF_H1I = 8384
_OFF_BIAS0 = 8448     # partition 0 only
_OFF_BIAS0S0 = 9472   # partition 0 only
_OFF_ONES = 10496     # partition 0 only
CB_COLS = 10528

# f32 const block [128, CF_COLS]: s0i | s1i (doubled initial states)
_OFF_S0I = 0
_OFF_S1I = 64
CF_COLS = 128


def _to_dev_mat(a):
    """[256 k, 1024 j] fp32 -> [128, 2048] with layout [p, (khalf, j)]."""
    assert a.shape == (256, 1024)
    return a.reshape(2, 128, 1024).transpose(1, 0, 2).reshape(128, 2048)


def _to_dev_state(a):
    """[32 b, 256 k] -> [128, 64] with layout [p, (khalf, b)]."""
    assert a.shape == (BSH, H)
    return a.T.reshape(2, 128, BSH).transpose(1, 0, 2).reshape(128, 2 * BSH)


def _prep_host(inputs):
    """Precompute device const blocks from the raw inputs."""
    W_ih_l0 = np.asarray(inputs["W_ih_l0"], np.float32)
    W_hh_l0 = np.asarray(inputs["W_hh_l0"], np.float32)
    b_ih_l0 = np.asarray(inputs["b_ih_l0"], np.float32)
    b_hh_l0 = np.asarray(inputs["b_hh_l0"], np.float32)
    W_ih_l1 = np.asarray(inputs["W_ih_l1"], np.float32)
    W_hh_l1 = np.asarray(inputs["W_hh_l1"], np.float32)
    b_ih_l1 = np.asarray(inputs["b_ih_l1"], np.float32)
    b_hh_l1 = np.asarray(inputs["b_hh_l1"], np.float32)
    W_out = np.asarray(inputs["W_out"], np.float32)
    b_out = np.asarray(inputs["b_out"], np.float32)
    z = np.asarray(inputs["z"], np.float32)

    # 0.5 scale folds the doubled stored state H=2h into each h-consumer.
    A0 = 0.5 * W_hh_l0.T[:, _PERM]                          # [256, 1024]
    Am = 0.5 * np.outer(W_out[0], W_ih_l0[:, 0])[:, _PERM]  # [256, 1024]
    A1i = 0.5 * W_ih_l1.T[:, _PERM]
    A1h = 0.5 * W_hh_l1.T[:, _PERM]
    bias0 = (b_ih_l0 + b_hh_l0 + b_out[0] * W_ih_l0[:, 0])[_PERM]  # t >= 1
    bias0_s0 = (b_ih_l0 + b_hh_l0)[_PERM]                          # t == 0
    bias1 = (b_ih_l1 + b_hh_l1)[_PERM]

    # One tanh serves all four gates with a constant 0.5 scale; the g gate
    # needs plain tanh, so its pre-activations are doubled here (exact in
    # bf16).
    for M in (A0, Am, A1i, A1h):
        M[:, 768:] *= 2.0
    for v in (bias0, bias0_s0, bias1):
        v[768:] *= 2.0

    per_core_cb = []
    per_core_cf = []
    for c in range(NCORES):
        zc = z[c * BSH:(c + 1) * BSH]  # [32, 256]
        zt = _to_dev_state(2.0 * zc)   # [128, 64]

        cb = np.zeros((128, CB_COLS), np.float32)
        cb[:, _OFF_A0:_OFF_A0 + 2048] = _to_dev_mat(A0)
        cb[:, _OFF_AM:_OFF_AM + 2048] = _to_dev_mat(Am)
        cb[:, _OFF_A1I:_OFF_A1I + 2048] = _to_dev_mat(A1i)
        cb[:, _OFF_A1H:_OFF_A1H + 2048] = _to_dev_mat(A1h)
        cb[:, _OFF_IDENT:_OFF_IDENT + 128] = np.eye(128, dtype=np.float32)
        cb[:, _OFF_H0I:_OFF_H0I + 64] = zt
        cb[:, _OFF_H1I:_OFF_H1I + 64] = zt
        cb[0, _OFF_BIAS0:_OFF_BIAS0 + 1024] = bias0
        cb[0, _OFF_BIAS0S0:_OFF_BIAS0S0 + 1024] = bias0_s0
        cb[0, _OFF_ONES:_OFF_ONES + BSH] = 1.0
        per_core_cb.append(cb.astype(BF16))

        cf = np.zeros((128, CF_COLS), np.float32)
        cf[:, _OFF_S0I:_OFF_S0I + 64] = zt
        cf[:, _OFF_S1I:_OFF_S1I + 64] = zt
        per_core_cf.append(cf)

    flags = {
        "has_bias0": bool(np.any(bias0 != 0)),
        "has_bias0_s0": bool(np.any(bias0_s0 != 0)),
        "has_bias1": bool(np.any(bias1 != 0)),
    }
    assert not flags["has_bias1"], "bias1 path not emitted in v2 kernel"
    return per_core_cb, per_core_cf, flags


def _build_program(t_steps, flags, reps=1):
    import concourse.bass as bass
    import concourse.mybir as mybir
    import concourse.tile as tile
    from concourse import bacc
    from contextlib import ExitStack, nullcontext

    f32 = mybir.dt.float32
    bf = mybir.dt.bfloat16
    ADD = mybir.AluOpType.add
    MULT = mybir.AluOpType.mult
    TANH = mybir.ActivationFunctionType.Tanh

    nc = bacc.Bacc("TRN2", target_bir_lowering=False, debug=False)

    dcb = nc.dram_tensor("cb", [128, CB_COLS], bf, kind="ExternalInput")
    dcf = nc.dram_tensor("cf", [128, CF_COLS], f32, kind="ExternalInput")
    dhist = nc.dram_tensor("hist", [128, t_steps * 2 * BSH], bf,
                           kind="ExternalOutput")

    with tile.TileContext(nc) as tc, ExitStack() as ctx:
        const = ctx.enter_context(tc.tile_pool(name="const", bufs=1))

        cbt = const.tile([128, CB_COLS], bf, tag="cb")
        nc.sync.dma_start(cbt[:], dcb[:, :])
        cft = const.tile([128, CF_COLS], f32, tag="cf")
        nc.sync.dma_start(cft[:], dcf[:, :])

        cbv = cbt[:]
        A0v = cbv[:, _OFF_A0:_OFF_A0 + 2048].rearrange("p (c j) -> p c j", c=2)
        Amv = cbv[:, _OFF_AM:_OFF_AM + 2048].rearrange("p (c j) -> p c j", c=2)
        A1iv = cbv[:, _OFF_A1I:_OFF_A1I + 2048].rearrange("p (c j) -> p c j", c=2)
        A1hv = cbv[:, _OFF_A1H:_OFF_A1H + 2048].rearrange("p (c j) -> p c j", c=2)
        ident = cbv[:, _OFF_IDENT:_OFF_IDENT + 128]
        h0iv = cbv[:, _OFF_H0I:_OFF_H0I + 64]
        h1iv = cbv[:, _OFF_H1I:_OFF_H1I + 64].rearrange("p (c b) -> p c b", c=2)
        bias0 = cbv[0:1, _OFF_BIAS0:_OFF_BIAS0 + 1024]
        bias0s0 = cbv[0:1, _OFF_BIAS0S0:_OFF_BIAS0S0 + 1024]
        ones = cbv[0:1, _OFF_ONES:_OFF_ONES + BSH]
        s0iv = cft[:][:, _OFF_S0I:_OFF_S0I + 64].rearrange(
            "p (c b) -> p c b", c=2)
        s1iv = cft[:][:, _OFF_S1I:_OFF_S1I + 64].rearrange(
            "p (c b) -> p c b", c=2)

        # mutable state: h0 bf16 (matmul stationary), S inside the U tiles
        # (slot 1; slot 0 is the per-step yg staging area). bf16 cell state
        # keeps rounding ~1e-3, far inside the loss tolerance, and gives the
        # 16-bit 2x DVE mode.
        h0 = const.tile([128, 2 * BSH], bf, tag="h0")
        U0 = const.tile([128, 4 * BSH], bf, tag="U0")
        U1 = const.tile([128, 4 * BSH], bf, tag="U1")
        h0v = h0[:].rearrange("p (c b) -> p c b", c=2)
        U0v = U0[:].rearrange("p (c u b) -> p c u b", c=2, u=2)
        U1v = U1[:].rearrange("p (c u b) -> p c u b", c=2, u=2)
        nc.vector.tensor_copy(h0[:], h0iv)
        nc.vector.tensor_copy(U0v[:, :, 1, :], s0iv)
        nc.vector.tensor_copy(U1v[:, :, 1, :], s1iv)

        hist = const.tile([128, t_steps * 2 * BSH], bf, tag="hist")
        histv = hist[:].rearrange("p (t c b) -> p t c b", t=t_steps, c=2)

        pg0 = ctx.enter_context(tc.tile_pool(name="pg0", bufs=2, space="PSUM"))
        pg1 = ctx.enter_context(tc.tile_pool(name="pg1", bufs=2, space="PSUM"))
        pyt = ctx.enter_context(tc.tile_pool(name="pyt", bufs=2, space="PSUM"))
        ypool = ctx.enter_context(tc.tile_pool(name="ypool", bufs=2))
        cellp = ctx.enter_context(tc.tile_pool(name="cellp", bufs=2))

        def mm_parts(gps, parts, start_i0=True, stop_last=True):
            n = len(parts)
            for i, (lh, rv, c) in enumerate(parts):
                for g in range(NG):
                    rhs = rv[0:1, g * GW:(g + 1) * GW] if c is None \
                        else rv[:, c, g * GW:(g + 1) * GW]
                    nc.tensor.matmul(
                        gps[32 * g:32 * (g + 1), :],
                        lh if c is None else lh[:, c, :],
                        rhs,
                        start=(start_i0 and i == 0),
                        stop=(stop_last and i == n - 1),
                        tile_position=(0, 32 * g),
                    )

        def cell(layer, gps, Uv, h_out_view):
            """Activation + transpose + fused-PQ LSTM cell update."""
            y = ypool.tile([128, 256], bf, tag=f"y{layer}")
            nc.scalar.activation(y[:], gps[:, :], TANH, scale=0.5)
            tp = pyt.tile([128, 256], bf, tag="tp")
            nc.tensor.transpose(tp[:, 0:128], y[:, 0:128], ident)
            nc.tensor.transpose(tp[:, 128:256], y[:, 128:256], ident)
            ytv = tp[:].rearrange("p (c q b) -> p c q b", c=2, q=NG)
            # stage yg into U slot 0 (vector ops may read only one PSUM
            # operand and PQ already reads Y[i,f] from PSUM); split per
            # k-half so the first copy runs under the second transpose
            nc.vector.tensor_copy(Uv[:, 0:1, 0, :], ytv[:, 0:1, 3, :])
            nc.vector.tensor_copy(Uv[:, 1:2, 0, :], ytv[:, 1:2, 3, :])
            PQ = cellp.tile([128, 4 * BSH], bf, tag=f"pq{layer}")
            PQv = PQ[:].rearrange("p (c u b) -> p c u b", c=2, u=2)
            # PQ[slot0] = (yi+1)*yg = p ; PQ[slot1] = (yf+1)*S = q
            nc.vector.scalar_tensor_tensor(PQv, ytv[:, :, 0:2, :], 1.0,
                                           Uv, ADD, MULT)
            sv = Uv[:, :, 1, :]
            nc.vector.scalar_tensor_tensor(sv, PQv[:, :, 1, :], 0.5,
                                           PQv[:, :, 0, :], MULT, ADD)
            tch = cellp.tile([128, 2 * BSH], bf, tag=f"tc{layer}")
            tcv = tch[:].rearrange("p (c b) -> p c b", c=2)
            nc.scalar.activation(tcv, sv, TANH, scale=0.5)
            # H' split per k-half so the downstream matmul's first k-half
            # wave can start while the second half is still being written.
            nc.vector.scalar_tensor_tensor(h_out_view[:, 0:1, :],
                                           ytv[:, 0:1, 2, :], 1.0,
                                           tcv[:, 0:1, :], ADD, MULT)
            nc.vector.scalar_tensor_tensor(h_out_view[:, 1:2, :],
                                           ytv[:, 1:2, 2, :], 1.0,
                                           tcv[:, 1:2, :], ADD, MULT)

        loop_cm = tc.For_i(0, reps, 1) if reps > 1 else nullcontext()
        with loop_cm:
          for t in range(t_steps):
            h1prev = histv[:, t - 1, :, :] if t > 0 else h1iv

            # layer-0 gates. bias+A0 depend only on h0(t-1): issued first,
            # they fill the PE while the previous step's cell-1 chain runs.
            # The Am (pred-feedback) waves need the fresh h1 and are the
            # only on-path g0 work.
            g0 = pg0.tile([128, 256], f32, tag="g0")
            parts = []
            if t > 0:
                if flags["has_bias0"]:
                    parts.append((ones, bias0, None))
            elif flags["has_bias0_s0"]:
                parts.append((ones, bias0s0, None))
            parts.append((h0v, A0v, 0))
            parts.append((h0v, A0v, 1))
            if t > 0:
                parts.append((h1prev, Amv, 0))
                parts.append((h1prev, Amv, 1))
            mm_parts(g0, parts)

            # layer-1 hh-part: same h1 dependency as Am; runs under act0.
            g1 = pg1.tile([128, 256], f32, tag="g1")
            mm_parts(g1, [(h1prev, A1hv, 0), (h1prev, A1hv, 1)],
                     start_i0=True, stop_last=False)

            cell(0, g0, U0v, h0v)

            # layer-1 ih-part (needs fresh h0), then cell.
            mm_parts(g1, [(h0v, A1iv, 0), (h0v, A1iv, 1)],
                     start_i0=False, stop_last=True)

            cell(1, g1, U1v, histv[:, t, :, :])

            if t % 32 == 31 or t == t_steps - 1:
                lo = (t // 32) * 32 * 2 * BSH
                hi = (t + 1) * 2 * BSH
                nc.sync.dma_start(dhist[:, lo:hi], hist[:][:, lo:hi])

    nc.compile()
    return nc


def _postprocess(results, inputs, t_steps):
    W_out = np.asarray(inputs["W_out"], np.float32)
    b_out = np.asarray(inputs["b_out"], np.float32)
    sequence = np.asarray(inputs["sequence"], np.float32)

    h1_all = np.empty((B, t_steps, H), np.float64)
    for c in range(NCORES):
        histd = np.asarray(results[c]["hist"]).astype(np.float32)
        # [128, t*2*32] -> [t, b, khalf, p] -> [t, b, 256]
        h1 = histd.reshape(128, t_steps, 2, BSH).transpose(1, 3, 2, 0)
        h1 = h1.reshape(t_steps, BSH, H) * 0.5  # undo doubling
        h1_all[c * BSH:(c + 1) * BSH] = h1.transpose(1, 0, 2)

    preds = h1_all @ W_out[0].astype(np.float64) + np.float64(b_out[0])  # [B, T]
    diff = sequence[:, :t_steps, 0].astype(np.float64) - preds
    loss = np.mean(diff * diff)
    return np.asarray(loss, dtype=np.float32)


def run(inputs, t_steps=T, trace=False):
    """Build + run on 8 cores; returns (loss, bass_results)."""
    from concourse.bass_utils import run_bass_kernel_spmd

    per_core_cb, per_core_cf, flags = _prep_host(inputs)
    nc = _build_program(t_steps, flags)
    in_maps = [{"cb": per_core_cb[c], "cf": per_core_cf[c]}
               for c in range(NCORES)]
    res = run_bass_kernel_spmd(nc, in_maps, list(range(NCORES)), trace=trace)
    loss = _postprocess(res.results, inputs, t_steps)
    return loss, res


def kernel(**inputs) -> np.ndarray:
    loss, _ = run(inputs, T, trace=False)
    return loss


# revision 10
# speedup vs baseline: 1.0594x; 1.0594x over previous
"""Trainium2 Bass kernel for nn_Decoder (2-layer LSTM autoregressive decoder).

Model (see reference): B=256 batch, T=512 steps, H=256 hidden, 2 LSTM layers,
scalar (F=1) autoregressive feedback through an output projection, final
output = scalar MSE loss vs `sequence`.

Strategy (v2)
-------------
- Data-parallel over batch: 8 NeuronCores x 32 batches each. Weights
  replicated. Zero cross-core communication; loss assembled on host.
- Per core, per step, gates are computed as [128, 256] PSUM tiles with
  partition = 32*gate + batch (gate order i,f,o,g) via 4-way column-tiled
  matmuls (M=32 stationary h, weights moving, all bf16).
- The scalar pred feedback is folded into a rank-1 matrix
  Am = 0.5*outer(W_out, W_ih_l0), so gates_l0(t) = H0(t-1)@A0 + H1(t-1)@Am
  + bias (a K=1 ones-row matmul).
- Wave order per step keeps only the truly h1(t-1)/h0(t)-dependent waves
  (Am, A1h, A1i) near the recurrence critical path; bias+A0 issue first and
  fill the PE under the previous step's cell-1 chain. (fp8 DoubleRow was
  evaluated to halve these waves but is invalid ISA for dst partitions
  32/96 - s3d3_mm_valid_dst_partition - so the waves stay bf16.)
- Cell chain per layer: one tanh act over all four gates
  (sigmoid(x) = (tanh(x/2)+1)/2, g-gate pre-activations doubled host-side,
  states kept doubled H=2h / S=2c), 2 PE transposes, then THREE DVE ops:
      copy  yg -> U[slot0]             (U[slot1] holds S persistently)
      PQ    = (Y[i,f] + 1) * U         (p = 2*sig(i)*g~ and q = 4*sig(f)*c
                                        fused into ONE scalar_tensor_tensor)
      S'    = 0.5*PQ[q] + PQ[p]        (written back into U[slot1])
  then tc = tanh(0.5*S') on the Act engine and H' = (yo+1)*tc.
- h1 history is written straight into a [128, T*64] bf16 SBUF buffer by the
  H' update op and DMA'd to HBM in chunks; preds + loss computed on host.
"""

import sys

import numpy as np

if "/opt/trn_rl_repo" not in sys.path:
    sys.path.insert(0, "/opt/trn_rl_repo")

import ml_dtypes

B, T, H = 256, 512, 256
NCORES = 8
BSH = B // NCORES  # 32 batches per core
NG = 4  # gates
GW = H  # gate width in j-dim (256)

BF16 = ml_dtypes.bfloat16

# device gate order: i, f, o, g (so sigmoid gates are partitions 0..95)
# pytorch row order in the 4H dim: i, f, g, o
_PERM = np.concatenate([
    np.arange(0, 256),       # i
    np.arange(256, 512),     # f
    np.arange(768, 1024),    # o
    np.arange(512, 768),     # g
])

# column offsets inside the big bf16 const block [128, CB_COLS]
_OFF_A0 = 0
_OFF_AM = 2048
_OFF_A1I = 4096
_OFF_A1H = 6144
_OFF_IDENT = 8192
_OFF_H0I = 8320
_OFF_H1I = 8384
_OFF_BIAS0 = 8448     # partition 0 only
_OFF_BIAS0S0 = 9472   # partition 0 only
_OFF_ONES = 10496     # partition 0 only
CB_COLS = 10528

# f32 const block [128, CF_COLS]: s0i | s1i (doubled initial states)
_OFF_S0I = 0
_OFF_S1I = 64
CF_COLS = 128


def _to_dev_mat(a):
    """[256 k, 1024 j] fp32 -> [128, 2048] with layout [p, (khalf, j)]."""
    assert a.shape == (256, 1024)
    return a.reshape(2, 128, 1024).transpose(1, 0, 2).reshape(128, 2048)


def _to_dev_state(a):
    """[32 b, 256 k] -> [128, 64] with layout [p, (khalf, b)]."""
    assert a.shape == (BSH, H)
    return a.T.reshape(2, 128, BSH).transpose(1, 0, 2).reshape(128, 2 * BSH)


def _prep_host(inputs):
    """Precompute device const blocks from the raw inputs."""
    W_ih_l0 = np.asarray(inputs["W_ih_l0"], np.float32)
    W_hh_l0 = np.asarray(inputs["W_hh_l0"], np.float32)
    b_ih_l0 = np.asarray(inputs["b_ih_l0"], np.float32)
    b_hh_l0 = np.asarray(inputs["b_hh_l0"], np.float32)
    W_ih_l1 = np.asarray(inputs["W_ih_l1"], np.float32)
    W_hh_l1 = np.asarray(inputs["W_hh_l1"], np.float32)
    b_ih_l1 = np.asarray(inputs["b_ih_l1"], np.float32)
    b_hh_l1 = np.asarray(inputs["b_hh_l1"], np.float32)
    W_out = np.asarray(inputs["W_out"], np.float32)
    b_out = np.asarray(inputs["b_out"], np.float32)
    z = np.asarray(inputs["z"], np.float32)

    # 0.5 scale folds the doubled stored state H=2h into each h-consumer.
    A0 = 0.5 * W_hh_l0.T[:, _PERM]                          # [256, 1024]
    Am = 0.5 * np.outer(W_out[0], W_ih_l0[:, 0])[:, _PERM]  # [256, 1024]
    A1i = 0.5 * W_ih_l1.T[:, _PERM]
    A1h = 0.5 * W_hh_l1.T[:, _PERM]
    bias0 = (b_ih_l0 + b_hh_l0 + b_out[0] * W_ih_l0[:, 0])[_PERM]  # t >= 1
    bias0_s0 = (b_ih_l0 + b_hh_l0)[_PERM]                          # t == 0
    bias1 = (b_ih_l1 + b_hh_l1)[_PERM]

    # One tanh serves all four gates with a constant 0.5 scale; the g gate
    # needs plain tanh, so its pre-activations are doubled here (exact in
    # bf16).
    for M in (A0, Am, A1i, A1h):
        M[:, 768:] *= 2.0
    for v in (bias0, bias0_s0, bias1):
        v[768:] *= 2.0

    per_core_cb = []
    per_core_cf = []
    for c in range(NCORES):
        zc = z[c * BSH:(c + 1) * BSH]  # [32, 256]
        zt = _to_dev_state(2.0 * zc)   # [128, 64]

        cb = np.zeros((128, CB_COLS), np.float32)
        cb[:, _OFF_A0:_OFF_A0 + 2048] = _to_dev_mat(A0)
        cb[:, _OFF_AM:_OFF_AM + 2048] = _to_dev_mat(Am)
        cb[:, _OFF_A1I:_OFF_A1I + 2048] = _to_dev_mat(A1i)
        cb[:, _OFF_A1H:_OFF_A1H + 2048] = _to_dev_mat(A1h)
        cb[:, _OFF_IDENT:_OFF_IDENT + 128] = np.eye(128, dtype=np.float32)
        cb[:, _OFF_H0I:_OFF_H0I + 64] = zt
        cb[:, _OFF_H1I:_OFF_H1I + 64] = zt
        cb[0, _OFF_BIAS0:_OFF_BIAS0 + 1024] = bias0
        cb[0, _OFF_BIAS0S0:_OFF_BIAS0S0 + 1024] = bias0_s0
        cb[0, _OFF_ONES:_OFF_ONES + BSH] = 1.0
        per_core_cb.append(cb.astype(BF16))

        cf = np.zeros((128, CF_COLS), np.float32)
        cf[:, _OFF_S0I:_OFF_S0I + 64] = zt
        cf[:, _OFF_S1I:_OFF_S1I + 64] = zt
        per_core_cf.append(cf)

    flags = {
        "has_bias0": bool(np.any(bias0 != 0)),
        "has_bias0_s0": bool(np.any(bias0_s0 != 0)),
        "has_bias1": bool(np.any(bias1 != 0)),
    }
    assert not flags["has_bias1"], "bias1 path not emitted in v2 kernel"
    return per_core_cb, per_core_cf, flags


def _build_program(t_steps, flags, reps=1):
    import concourse.bass as bass
    import concourse.mybir as mybir
    import concourse.tile as tile
    from concourse import bacc
    from contextlib import ExitStack, nullcontext

    f32 = mybir.dt.float32
    bf = mybir.dt.bfloat16
    ADD = mybir.AluOpType.add
    MULT = mybir.AluOpType.mult
    TANH = mybir.ActivationFunctionType.Tanh

    nc = bacc.Bacc("TRN2", target_bir_lowering=False, debug=False)

    dcb = nc.dram_tensor("cb", [128, CB_COLS], bf, kind="ExternalInput")
    dcf = nc.dram_tensor("cf", [128, CF_COLS], f32, kind="ExternalInput")
    dhist = nc.dram_tensor("hist", [128, t_steps * 2 * BSH], bf,
                           kind="ExternalOutput")

    with tile.TileContext(nc) as tc, ExitStack() as ctx:
        const = ctx.enter_context(tc.tile_pool(name="const", bufs=1))

        cbt = const.tile([128, CB_COLS], bf, tag="cb")
        nc.sync.dma_start(cbt[:], dcb[:, :])
        cft = const.tile([128, CF_COLS], f32, tag="cf")
        nc.sync.dma_start(cft[:], dcf[:, :])

        cbv = cbt[:]
        A0v = cbv[:, _OFF_A0:_OFF_A0 + 2048].rearrange("p (c j) -> p c j", c=2)
        Amv = cbv[:, _OFF_AM:_OFF_AM + 2048].rearrange("p (c j) -> p c j", c=2)
        A1iv = cbv[:, _OFF_A1I:_OFF_A1I + 2048].rearrange("p (c j) -> p c j", c=2)
        A1hv = cbv[:, _OFF_A1H:_OFF_A1H + 2048].rearrange("p (c j) -> p c j", c=2)
        ident = cbv[:, _OFF_IDENT:_OFF_IDENT + 128]
        h0iv = cbv[:, _OFF_H0I:_OFF_H0I + 64]
        h1iv = cbv[:, _OFF_H1I:_OFF_H1I + 64].rearrange("p (c b) -> p c b", c=2)
        bias0 = cbv[0:1, _OFF_BIAS0:_OFF_BIAS0 + 1024]
        bias0s0 = cbv[0:1, _OFF_BIAS0S0:_OFF_BIAS0S0 + 1024]
        ones = cbv[0:1, _OFF_ONES:_OFF_ONES + BSH]
        s0iv = cft[:][:, _OFF_S0I:_OFF_S0I + 64].rearrange(
            "p (c b) -> p c b", c=2)
        s1iv = cft[:][:, _OFF_S1I:_OFF_S1I + 64].rearrange(
            "p (c b) -> p c b", c=2)

        # mutable state: h0 bf16 (matmul stationary), S inside the U tiles
        # (slot 1; slot 0 is the per-step yg staging area). bf16 cell state
        # keeps rounding ~1e-3, far inside the loss tolerance, and gives the
        # 16-bit 2x DVE mode.
        h0 = const.tile([128, 2 * BSH], bf, tag="h0")
        U0 = const.tile([128, 4 * BSH], bf, tag="U0")
        U1 = const.tile([128, 4 * BSH], bf, tag="U1")
        h0v = h0[:].rearrange("p (c b) -> p c b", c=2)
        U0v = U0[:].rearrange("p (c u b) -> p c u b", c=2, u=2)
        U1v = U1[:].rearrange("p (c u b) -> p c u b", c=2, u=2)
        nc.vector.tensor_copy(h0[:], h0iv)
        nc.vector.tensor_copy(U0v[:, :, 1, :], s0iv)
        nc.vector.tensor_copy(U1v[:, :, 1, :], s1iv)

        hist = const.tile([128, t_steps * 2 * BSH], bf, tag="hist")
        histv = hist[:].rearrange("p (t c b) -> p t c b", t=t_steps, c=2)

        pg0 = ctx.enter_context(tc.tile_pool(name="pg0", bufs=2, space="PSUM"))
        pg1 = ctx.enter_context(tc.tile_pool(name="pg1", bufs=2, space="PSUM"))
        pyt = ctx.enter_context(tc.tile_pool(name="pyt", bufs=2, space="PSUM"))
        ypool = ctx.enter_context(tc.tile_pool(name="ypool", bufs=2))
        cellp = ctx.enter_context(tc.tile_pool(name="cellp", bufs=2))

        def mm_parts(gps, parts, start_i0=True, stop_last=True):
            n = len(parts)
            for i, (lh, rv, c) in enumerate(parts):
                for g in range(NG):
                    rhs = rv[0:1, g * GW:(g + 1) * GW] if c is None \
                        else rv[:, c, g * GW:(g + 1) * GW]
                    nc.tensor.matmul(
                        gps[32 * g:32 * (g + 1), :],
                        lh if c is None else lh[:, c, :],
                        rhs,
                        start=(start_i0 and i == 0),
                        stop=(stop_last and i == n - 1),
                        tile_position=(0, 32 * g),
                    )

        def cell(layer, gps, Uv, h_out_view):
            """Activation + transpose + fused-PQ LSTM cell update."""
            y = ypool.tile([128, 256], bf, tag=f"y{layer}")
            nc.scalar.activation(y[:], gps[:, :], TANH, scale=0.5)
            tp = pyt.tile([128, 256], bf, tag="tp")
            nc.tensor.transpose(tp[:, 0:128], y[:, 0:128], ident)
            nc.tensor.transpose(tp[:, 128:256], y[:, 128:256], ident)
            ytv = tp[:].rearrange("p (c q b) -> p c q b", c=2, q=NG)
            # stage yg into U slot 0 (vector ops may read only one PSUM
            # operand and PQ already reads Y[i,f] from PSUM)
            nc.vector.tensor_copy(Uv[:, :, 0, :], ytv[:, :, 3, :])
            PQ = cellp.tile([128, 4 * BSH], bf, tag=f"pq{layer}")
            PQv = PQ[:].rearrange("p (c u b) -> p c u b", c=2, u=2)
            # PQ[slot0] = (yi+1)*yg = p ; PQ[slot1] = (yf+1)*S = q
            nc.vector.scalar_tensor_tensor(PQv, ytv[:, :, 0:2, :], 1.0,
                                           Uv, ADD, MULT)
            sv = Uv[:, :, 1, :]
            nc.vector.scalar_tensor_tensor(sv, PQv[:, :, 1, :], 0.5,
                                           PQv[:, :, 0, :], MULT, ADD)
            tch = cellp.tile([128, 2 * BSH], bf, tag=f"tc{layer}")
            tcv = tch[:].rearrange("p (c b) -> p c b", c=2)
            nc.scalar.activation(tcv, sv, TANH, scale=0.5)
            # H' split per k-half so the downstream matmul's first k-half
            # wave can start while the second half is still being written.
            nc.vector.scalar_tensor_tensor(h_out_view[:, 0:1, :],
                                           ytv[:, 0:1, 2, :], 1.0,
                                           tcv[:, 0:1, :], ADD, MULT)
            nc.vector.scalar_tensor_tensor(h_out_view[:, 1:2, :],
                                           ytv[:, 1:2, 2, :], 1.0,
                                           tcv[:, 1:2, :], ADD, MULT)

        loop_cm = tc.For_i(0, reps, 1) if reps > 1 else nullcontext()
        with loop_cm:
          for t in range(t_steps):
            h1prev = histv[:, t - 1, :, :] if t > 0 else h1iv

            # layer-0 gates. bias+A0 depend only on h0(t-1): issued first,
            # they fill the PE while the previous step's cell-1 chain runs.
            # The Am (pred-feedback) waves need the fresh h1 and are the
            # only on-path g0 work.
            g0 = pg0.tile([128, 256], f32, tag="g0")
            parts = []
            if t > 0:
                if flags["has_bias0"]:
                    parts.append((ones, bias0, None))
            elif flags["has_bias0_s0"]:
                parts.append((ones, bias0s0, None))
            parts.append((h0v, A0v, 0))
            parts.append((h0v, A0v, 1))
            if t > 0:
                parts.append((h1prev, Amv, 0))
                parts.append((h1prev, Amv, 1))
            mm_parts(g0, parts)

            # layer-1 hh-part: same h1 dependency as Am; runs under act0.
            g1 = pg1.tile([128, 256], f32, tag="g1")
            mm_parts(g1, [(h1prev, A1hv, 0), (h1prev, A1hv, 1)],
                     start_i0=True, stop_last=False)

            cell(0, g0, U0v, h0v)

            # layer-1 ih-part (needs fresh h0), then cell.
            mm_parts(g1, [(h0v, A1iv, 0), (h0v, A1iv, 1)],
                     start_i0=False, stop_last=True)

            cell(1, g1, U1v, histv[:, t, :, :])

            if t % 32 == 31 or t == t_steps - 1:
                lo = (t // 32) * 32 * 2 * BSH
                hi = (t + 1) * 2 * BSH
                nc.sync.dma_start(dhist[:, lo:hi], hist[:][:, lo:hi])

    nc.compile()
    return nc


def _postprocess(results, inputs, t_steps):
    W_out = np.asarray(inputs["W_out"], np.float32)
    b_out = np.asarray(inputs["b_out"], np.float32)
    sequence = np.asarray(inputs["sequence"], np.float32)

    h1_all = np.empty((B, t_steps, H), np.float64)
    for c in range(NCORES):
        histd = np.asarray(results[c]["hist"]).astype(np.float32)
        # [128, t*2*32] -> [t, b, khalf, p] -> [t, b, 256]
        h1 = histd.reshape(128, t_steps, 2, BSH).transpose(1, 3, 2, 0)
        h1 = h1.reshape(t_steps, BSH, H) * 0.5  # undo doubling
        h1_all[c * BSH:(c + 1) * BSH] = h1.transpose(1, 0, 2)

    preds = h1_all @ W_out[0].astype(np.float64) + np.float64(b_out[0])  # [B, T]
    diff = sequence[:, :t_steps, 0].astype(np.float64) - preds
    loss = np.mean(diff * diff)
    return np.asarray(loss, dtype=np.float32)


def run(inputs, t_steps=T, trace=False):
    """Build + run on 8 cores; returns (loss, bass_results)."""
    from concourse.bass_utils import run_bass_kernel_spmd

    per_core_cb, per_core_cf, flags = _prep_host(inputs)
    nc = _build_program(t_steps, flags)
    in_maps = [{"cb": per_core_cb[c], "cf": per_core_cf[c]}
               for c in range(NCORES)]
    res = run_bass_kernel_spmd(nc, in_maps, list(range(NCORES)), trace=trace)
    loss = _postprocess(res.results, inputs, t_steps)
    return loss, res


def kernel(**inputs) -> np.ndarray:
    loss, _ = run(inputs, T, trace=False)
    return loss
